# revision 11
# baseline (speedup 1.0000x reference)
"""Trainium2 Bass kernel for the CogVideoX attention-processor block.

Sharding (8 cores): core c handles batch c//4 and head-group c%4 (4 of 16
heads). Each core computes QKV projections for its 256 features over the full
(text+video) sequence, per-head LayerNorm + RoPE, full softmax attention for
its 4 heads, and a partial output projection [S, 1024]. The host sums the 4
partials per batch and adds the output bias.

Device-side layout notes:
  - Sequence is laid out video-first ([video 2048 | text 226 | pad 30] = 2304)
    so the RoPE region is exactly the first four 512-wide i-blocks.
  - q/k are produced *transposed* ([dh, S], feature-on-partition) so the
    scores matmul s_T = k @ q^T and the PV matmul contract on partitions.
  - LayerNorm mean-centering is folded into wq/wk on the host (C = I - 1/64
    per head block); on-device LN is sumsq (ones-matmul) -> sqrt -> recip ->
    outer-product broadcast -> multiply.
  - Softmax skips max-subtraction: LN gives q,k exact norm sqrt(64), so
    |scores| <= 8 after the 1/8 scale; exp is computed directly on ACT.
  - The PV matmul uses v augmented with a ones column, yielding the softmax
    denominator as row 64 of the output for free. Padding rows of v_aug are
    zeroed, which masks the 30 pad key positions exactly.
"""

import numpy as np
import ml_dtypes

B, S_TXT, S_VID, D, H = 2, 226, 2048, 1024, 16
DH = 64
S = S_TXT + S_VID            # 2274
SP = 2304                    # padded to 18*128
HPC = 4                      # heads per core
FPC = HPC * DH               # features per core (256)
LN_EPS = 1e-5
NMC = D // 128               # 8 contraction chunks
NJC = SP // 128              # 18 key chunks
IBLKS = [(0, 512), (512, 512), (1024, 512), (1536, 512), (2048, 256)]
N_VID_IB = 4                 # i-blocks 0..3 are exactly the video positions
PAD_ROWS = SP - S            # 30
VALID_LAST = 128 - PAD_ROWS  # 98 valid rows in the last key chunk

_BF16 = ml_dtypes.bfloat16
_prog_cache = {}
_last = {}


def _last_run_for_profile(inputs=None):
    """Re-run the last kernel invocation with NTFF tracing (test harness aid)."""
    if "nc" not in _last:
        return None
    from concourse.bass_utils import run_bass_kernel_spmd
    return run_bass_kernel_spmd(_last["nc"], _last["in_maps"], list(range(8)),
                                trace=True)


def _build_nc(apply_w_q, apply_b_q, apply_w_k, apply_b_k):
    import concourse.bass as bass
    import concourse.mybir as mybir
    import concourse.tile as tile
    from concourse import bacc
    from concourse.bass import ts

    f32 = mybir.dt.float32
    bf16 = mybir.dt.bfloat16
    Exp = mybir.ActivationFunctionType.Exp
    Sqrt = mybir.ActivationFunctionType.Sqrt
    add_op = mybir.AluOpType.add
    mult_op = mybir.AluOpType.mult

    nc = bacc.Bacc()

    hT_d = nc.dram_tensor("hT", [D, SP], bf16, kind="ExternalInput")
    wqT_d = nc.dram_tensor("wqT", [D, FPC], bf16, kind="ExternalInput")
    wkT_d = nc.dram_tensor("wkT", [D, FPC], bf16, kind="ExternalInput")
    wvT_d = nc.dram_tensor("wvT", [D, FPC], bf16, kind="ExternalInput")
    woT_d = nc.dram_tensor("woT", [FPC, D], bf16, kind="ExternalInput")
    bq_d = nc.dram_tensor("bq", [128, 2], f32, kind="ExternalInput")
    bk_d = nc.dram_tensor("bk", [128, 2], f32, kind="ExternalInput")
    bv_d = nc.dram_tensor("bv", [128, FPC], f32, kind="ExternalInput")
    cosT_d = nc.dram_tensor("cosT", [128, S_VID], f32, kind="ExternalInput")
    sinT_d = nc.dram_tensor("sinT", [128, S_VID], f32, kind="ExternalInput")
    rope_d = nc.dram_tensor("ropeT", [128, 128], bf16, kind="ExternalInput")
    onesb_d = nc.dram_tensor("onesblk", [128, 2], bf16, kind="ExternalInput")
    esel_d = nc.dram_tensor("esel", [2, 128], f32, kind="ExternalInput")
    ones64_d = nc.dram_tensor("ones64", [1, 64], f32, kind="ExternalInput")
    lnc_d = nc.dram_tensor("lncols", [128, 4], f32, kind="ExternalInput")
    out_d = nc.dram_tensor("out", [SP, D], f32, kind="ExternalOutput")

    with tile.TileContext(nc) as tc:
        from contextlib import ExitStack
        with ExitStack() as ctx:
            consts = ctx.enter_context(tc.tile_pool(name="consts", bufs=1))
            big = ctx.enter_context(tc.tile_pool(name="bigps", bufs=4, space="PSUM"))
            vps = ctx.enter_context(tc.tile_pool(name="vps", bufs=2, space="PSUM"))
            ops = ctx.enter_context(tc.tile_pool(name="ops", bufs=2, space="PSUM"))
            sbA = ctx.enter_context(tc.tile_pool(name="sbA", bufs=3))
            sbS = ctx.enter_context(tc.tile_pool(name="sbS", bufs=4))
            sbP = ctx.enter_context(tc.tile_pool(name="sbP", bufs=4))
            sbO = ctx.enter_context(tc.tile_pool(name="sbO", bufs=3))

            # ---- resident constants / weights / state ----
            h_sb = consts.tile([128, NMC, SP], bf16)
            wq_sb = consts.tile([128, NMC, FPC], bf16)
            wk_sb = consts.tile([128, NMC, FPC], bf16)
            wv_sb = consts.tile([128, NMC, FPC], bf16)
            wo_sb = consts.tile([128, 2, D], bf16)
            cos_sb = consts.tile([128, S_VID], f32)
            sin_sb = consts.tile([128, S_VID], f32)
            rope_sb = consts.tile([128, 128], bf16)
            onesb_sb = consts.tile([128, 2], bf16)
            esel_sb = consts.tile([2, 128], f32)
            ones64_sb = consts.tile([1, 64], f32)
            bq_sb = consts.tile([128, 2], f32)
            bk_sb = consts.tile([128, 2], f32)
            bv_sb = consts.tile([128, HPC, DH], f32)
            ln_sb = consts.tile([128, 4], f32)
            eps_sb = consts.tile([2, 1], f32)
            qT = consts.tile([128, 2, SP], bf16)
            kT = consts.tile([128, 2, SP], bf16)
            vaug = consts.tile([128, NJC, HPC, 66], bf16)
            oT = consts.tile([128, 2, SP], bf16)

            nc.sync.dma_start(wq_sb, wqT_d[:, :].rearrange("(mc p) d -> p mc d", p=128))
            nc.sync.dma_start(wk_sb, wkT_d[:, :].rearrange("(mc p) d -> p mc d", p=128))
            nc.sync.dma_start(wv_sb, wvT_d[:, :].rearrange("(mc p) d -> p mc d", p=128))
            nc.sync.dma_start(wo_sb, woT_d[:, :].rearrange("(c p) n -> p c n", p=128))
            nc.sync.dma_start(cos_sb, cosT_d[:, :])
            nc.sync.dma_start(sin_sb, sinT_d[:, :])
            nc.sync.dma_start(rope_sb, rope_d[:, :])
            nc.sync.dma_start(onesb_sb, onesb_d[:, :])
            nc.sync.dma_start(esel_sb, esel_d[:, :])
            nc.sync.dma_start(ones64_sb, ones64_d[:, :])
            nc.sync.dma_start(bq_sb, bq_d[:, :])
            nc.sync.dma_start(bk_sb, bk_d[:, :])
            nc.sync.dma_start(ln_sb, lnc_d[:, :])
            nc.sync.dma_start(bv_sb, bv_d[:, :].rearrange("p (h d) -> p h d", d=DH))
            for m in range(NMC):
                nc.sync.dma_start(h_sb[:, m, :], hT_d[ts(m, 128), :])
            nc.vector.memset(eps_sb, LN_EPS)

            # vaug ones/padding prep (partition bases must be 32-aligned)
            nc.vector.memset(vaug[:, :, :, 64:66], 0.0)
            nc.vector.memset(vaug[96:128, NJC - 1, :, 0:64], 0.0)
            nc.vector.memset(vaug[:, 0:NJC - 1, :, 64:65], 1.0)
            nc.vector.memset(vaug[0:96, NJC - 1, :, 64:65], 1.0)
            nc.vector.memset(vaug[96:VALID_LAST, NJC - 1, :, 64:65], 1.0)

            # ---- phase 1a: v for all key chunks ----
            for ib, (i0, W) in enumerate(IBLKS):
                for sc in range(W // 128):
                    jc = i0 // 128 + sc
                    ps_v = vps.tile([128, FPC], f32, tag="v")
                    for m in range(NMC):
                        nc.tensor.matmul(ps_v, h_sb[:, m, i0 + 128 * sc:i0 + 128 * (sc + 1)],
                                         wv_sb[:, m, :], start=(m == 0), stop=(m == NMC - 1))
                    if jc == NJC - 1:
                        # split at 32-aligned partition bases; rows 98+ stay 0
                        nc.vector.tensor_add(
                            vaug[0:96, jc, :, 0:64],
                            ps_v[0:96].rearrange("p (h d) -> p h d", d=DH),
                            bv_sb[0:96])
                        nc.vector.tensor_add(
                            vaug[96:VALID_LAST, jc, :, 0:64],
                            ps_v[96:VALID_LAST].rearrange("p (h d) -> p h d", d=DH),
                            bv_sb[96:VALID_LAST])
                    else:
                        nc.vector.tensor_add(
                            vaug[:, jc, :, 0:64],
                            ps_v[:].rearrange("p (h d) -> p h d", d=DH),
                            bv_sb[:])

            # ---- phase 1b/2: per chunk: q/k production then attention ----
            def qk_block(c, ib, i0, W, wT_sb, b_sb, dstT, apply_w, apply_b, wcol, bcol):
                ps = big.tile([128, 512], f32, tag="big")
                for m in range(NMC):
                    nc.tensor.matmul(ps[:, 0:W], wT_sb[:, m, ts(c, 128)],
                                     h_sb[:, m, i0:i0 + W],
                                     start=(m == 0), stop=(m == NMC - 1))
                xc = sbA.tile([128, 512], bf16, tag="xc", name="xc")[:, 0:W]
                nc.vector.tensor_scalar(xc, ps[:, 0:W], b_sb[:, c:c + 1], None, add_op)
                sq = sbA.tile([128, 512], bf16, tag="sq", name="sq")[:, 0:W]
                nc.vector.tensor_mul(sq, xc, xc)
                st = big.tile([128, 512], f32, tag="big")
                nc.tensor.matmul(st[0:2, 0:W], onesb_sb, sq, start=True, stop=True)
                sd = sbS.tile([2, 512], f32, tag="sd", name="sd")[:, 0:W]
                nc.scalar.activation(sd, st[0:2, 0:W], Sqrt, scale=1.0 / DH, bias=eps_sb)
                rstd = sbS.tile([2, 512], f32, tag="rstd", name="rstd")[:, 0:W]
                nc.vector.reciprocal(rstd, sd)
                rb = big.tile([128, 512], f32, tag="big")
                nc.tensor.matmul(rb[:, 0:W], esel_sb, rstd, start=True, stop=True)
                is_rope = ib < N_VID_IB
                need_wb = apply_w or apply_b
                if is_rope or need_wb:
                    xn = sbA.tile([128, 512], bf16, tag="xn", name="xn")[:, 0:W]
                else:
                    xn = dstT[:, c, i0:i0 + W]
                nc.vector.tensor_mul(xn, xc, rb[:, 0:W])
                if need_wb:
                    tgt = sbA.tile([128, 512], bf16, tag="xw", name="xw")[:, 0:W] if is_rope \
                        else dstT[:, c, i0:i0 + W]
                    if apply_w and apply_b:
                        nc.vector.tensor_scalar(tgt, xn, wcol, bcol, mult_op, add_op)
                    elif apply_w:
                        nc.vector.tensor_scalar(tgt, xn, wcol, None, mult_op)
                    else:
                        nc.vector.tensor_scalar(tgt, xn, bcol, None, add_op)
                    xn = tgt
                if is_rope:
                    rp = big.tile([128, 512], f32, tag="big")
                    nc.tensor.matmul(rp[:, 0:W], rope_sb, xn, start=True, stop=True)
                    t1 = sbA.tile([128, 512], f32, tag="t1", name="t1")[:, 0:W]
                    nc.vector.tensor_mul(t1, xn, cos_sb[:, i0:i0 + W])
                    t2 = sbA.tile([128, 512], f32, tag="t2", name="t2")[:, 0:W]
                    nc.vector.tensor_mul(t2, rp[:, 0:W], sin_sb[:, i0:i0 + W])
                    nc.vector.tensor_add(dstT[:, c, i0:i0 + W], t1, t2)

            def attn_block(c, ib, i0, W):
                o_ps = [ops.tile([65, 512], f32, tag="o", name="o_ps")[:, 0:W] for _ in range(2)]
                for jc in range(NJC):
                    for hh in range(2):
                        p0 = 64 * hh
                        s_ps = big.tile([128, 512], f32, tag="big")
                        nc.tensor.matmul(s_ps[:, 0:W],
                                         kT[p0:p0 + 64, c, ts(jc, 128)],
                                         qT[p0:p0 + 64, c, i0:i0 + W],
                                         start=True, stop=True,
                                         tile_position=(p0, 0))
                        p_sb = sbP.tile([128, 512], bf16, tag="p", name="p_sb")[:, 0:W]
                        nc.scalar.activation(p_sb, s_ps[:, 0:W], Exp, scale=0.125)
                        nc.tensor.matmul(o_ps[hh], vaug[:, jc, 2 * c + hh, 0:65], p_sb,
                                         start=(jc == 0), stop=(jc == NJC - 1))
                for hh in range(2):
                    osb = sbA.tile([65, 512], f32, tag="osb", name="osb")[:, 0:W]
                    nc.vector.tensor_copy(osb, o_ps[hh])
                    rden = sbS.tile([1, 512], f32, tag="rden", name="rden")[:, 0:W]
                    nc.vector.reciprocal(rden, osb[64:65])
                    db = big.tile([128, 512], f32, tag="big")
                    nc.tensor.matmul(db[0:64, 0:W], ones64_sb, rden, start=True, stop=True)
                    p0 = 64 * hh
                    nc.vector.tensor_mul(oT[p0:p0 + 64, c, i0:i0 + W],
                                         osb[0:64], db[0:64, 0:W])

            for c in range(2):
                for ib, (i0, W) in enumerate(IBLKS):
                    qk_block(c, ib, i0, W, wq_sb, bq_sb, qT, apply_w_q, apply_b_q,
                             ln_sb[:, 0:1], ln_sb[:, 1:2])
                    qk_block(c, ib, i0, W, wk_sb, bk_sb, kT, apply_w_k, apply_b_k,
                             ln_sb[:, 2:3], ln_sb[:, 3:4])
                for ib, (i0, W) in enumerate(IBLKS):
                    attn_block(c, ib, i0, W)

            # ---- phase 3: output projection (partial) ----
            for ic in range(NJC):
                ps0 = big.tile([128, 512], f32, tag="big")
                ps1 = big.tile([128, 512], f32, tag="big")
                for c in range(2):
                    nc.tensor.matmul(ps0, oT[:, c, ts(ic, 128)], wo_sb[:, c, 0:512],
                                     start=(c == 0), stop=(c == 1))
                    nc.tensor.matmul(ps1, oT[:, c, ts(ic, 128)], wo_sb[:, c, 512:1024],
                                     start=(c == 0), stop=(c == 1))
                ob = sbO.tile([128, D], f32, tag="ob")
                nc.vector.tensor_copy(ob[:, 0:512], ps0)
                nc.vector.tensor_copy(ob[:, 512:1024], ps1)
                nc.sync.dma_start(out_d[ts(ic, 128), :], ob)

    nc.finalize()
    return nc


def _get_prog(flags):
    if flags not in _prog_cache:
        _prog_cache[flags] = _build_nc(*flags)
    return _prog_cache[flags]


def _rope_mat():
    P = np.zeros((DH, DH), np.float32)
    for m in range(DH // 2):
        P[2 * m, 2 * m + 1] = -1.0
        P[2 * m + 1, 2 * m] = 1.0
    # lhsT = P^T, block-diagonal over the two heads in a partition chunk
    PT = P.T
    R = np.zeros((128, 128), np.float32)
    R[0:64, 0:64] = PT
    R[64:128, 64:128] = PT
    return R


def kernel(hidden_states, encoder_hidden_states, cos, sin, wq, bq, wk, bk,
           wv, bv, wo, bo, lnq_w, lnq_b, lnk_w, lnk_b):
    from concourse.bass_utils import run_bass_kernel_spmd

    f32 = np.float32
    hs = np.asarray(hidden_states, f32)
    ehs = np.asarray(encoder_hidden_states, f32)
    cos = np.asarray(cos, f32)
    sin = np.asarray(sin, f32)
    wq = np.asarray(wq, f32); bq = np.asarray(bq, f32)
    wk = np.asarray(wk, f32); bk = np.asarray(bk, f32)
    wv = np.asarray(wv, f32); bv = np.asarray(bv, f32)
    wo = np.asarray(wo, f32); bo = np.asarray(bo, f32)
    lnq_w = np.asarray(lnq_w, f32); lnq_b = np.asarray(lnq_b, f32)
    lnk_w = np.asarray(lnk_w, f32); lnk_b = np.asarray(lnk_b, f32)

    flags = (not np.all(lnq_w == 1.0), bool(np.any(lnq_b)),
             not np.all(lnk_w == 1.0), bool(np.any(lnk_b)))
    nc = _get_prog(flags)

    # fold LN mean-centering into wq/wk (per 64-row head block)
    def center_rows(w):
        shp = w.shape
        w64 = w.astype(np.float64).reshape(H, DH, -1)
        w64 = w64 - w64.mean(axis=1, keepdims=True)
        return w64.reshape(shp).astype(f32)

    wq_c = center_rows(wq)
    wk_c = center_rows(wk)
    bq_c = center_rows(bq.reshape(D, 1)).reshape(D)
    bk_c = center_rows(bk.reshape(D, 1)).reshape(D)

    rope = _rope_mat().astype(_BF16)
    onesblk = np.zeros((128, 2), _BF16)
    onesblk[0:64, 0] = 1.0
    onesblk[64:128, 1] = 1.0
    esel = np.zeros((2, 128), f32)
    esel[0, 0:64] = 1.0
    esel[1, 64:128] = 1.0
    ones64 = np.ones((1, 64), f32)
    cosT = np.concatenate([cos.T, cos.T], axis=0).astype(f32).copy()
    sinT = np.concatenate([sin.T, sin.T], axis=0).astype(f32).copy()
    lncols = np.stack([np.tile(lnq_w, 2), np.tile(lnq_b, 2),
                       np.tile(lnk_w, 2), np.tile(lnk_b, 2)], axis=1).astype(f32)

    in_maps = []
    for core in range(8):
        b = core // 4
        g = core % 4
        F = slice(FPC * g, FPC * (g + 1))
        h_full = np.concatenate([hs[b], ehs[b]], axis=0)      # video-first [S, D]
        hT = np.zeros((D, SP), _BF16)
        hT[:, :S] = h_full.T.astype(_BF16)
        in_maps.append({
            "hT": hT,
            "wqT": np.ascontiguousarray(wq_c[F].T).astype(_BF16),
            "wkT": np.ascontiguousarray(wk_c[F].T).astype(_BF16),
            "wvT": np.ascontiguousarray(wv[F].T).astype(_BF16),
            "woT": np.ascontiguousarray(wo[:, F].T).astype(_BF16),
            "bq": np.ascontiguousarray(bq_c[F].reshape(2, 128).T).astype(f32),
            "bk": np.ascontiguousarray(bk_c[F].reshape(2, 128).T).astype(f32),
            "bv": np.ascontiguousarray(np.tile(bv[F], (128, 1))).astype(f32),
            "cosT": cosT, "sinT": sinT,
            "ropeT": rope, "onesblk": onesblk, "esel": esel, "ones64": ones64,
            "lncols": lncols,
        })

    _last["nc"] = nc
    _last["in_maps"] = in_maps
    res = run_bass_kernel_spmd(nc, in_maps, list(range(8)))
    outs = np.zeros((B, SP, D), f32)
    for core in range(8):
        outs[core // 4] += np.asarray(res.results[core]["out"], f32)
    outs += bo[None, None, :]
    video = np.ascontiguousarray(outs[:, :S_VID, :])
    text = np.ascontiguousarray(outs[:, S_VID:S, :])
    return video, text


# revision 16
# speedup vs baseline: 1.3255x; 1.3255x over previous
"""Trainium2 Bass kernel for the CogVideoX attention-processor block.

Sharding (8 cores): core c handles batch c//4 and head-group c%4 (4 of 16
heads). Each core computes QKV projections for its 256 features over the full
(text+video) sequence, per-head LayerNorm + RoPE, full softmax attention for
its 4 heads, and a partial output projection [S, 1024]. The host sums the 4
partials per batch and adds the output bias.

Device-side layout notes:
  - Sequence is laid out video-first ([video 2048 | text 226 | pad 30] = 2304)
    so the RoPE region is exactly the first four 512-wide i-blocks.
  - q/k are produced *transposed* ([dh, S], feature-on-partition) so the
    scores matmul s_T = k @ q^T and the PV matmul contract on partitions.
  - LayerNorm mean-centering is folded into wq/wk on the host (C = I - 1/64
    per head block); on-device LN is sumsq (ones-matmul) -> sqrt -> recip ->
    outer-product broadcast -> multiply.
  - Softmax skips max-subtraction: LN gives q,k exact norm sqrt(64), so
    |scores| <= 8 after the 1/8 scale; exp is computed directly on ACT.
  - The PV matmul uses v augmented with a ones column, yielding the softmax
    denominator as row 64 of the output for free. Padding rows of v_aug are
    zeroed, which masks the 30 pad key positions exactly.
"""

import numpy as np
import ml_dtypes

B, S_TXT, S_VID, D, H = 2, 226, 2048, 1024, 16
DH = 64
S = S_TXT + S_VID            # 2274
SP = 2304                    # padded to 18*128
HPC = 4                      # heads per core
FPC = HPC * DH               # features per core (256)
LN_EPS = 1e-5
NMC = D // 128               # 8 contraction chunks
NJC = SP // 128              # 18 key chunks
IBLKS = [(0, 512), (512, 512), (1024, 512), (1536, 512), (2048, 256)]
N_VID_IB = 4                 # i-blocks 0..3 are exactly the video positions
PAD_ROWS = SP - S            # 30
VALID_LAST = 128 - PAD_ROWS  # 98 valid rows in the last key chunk

_BF16 = ml_dtypes.bfloat16
_prog_cache = {}
_last = {}


def _last_run_for_profile(inputs=None):
    """Re-run the last kernel invocation with NTFF tracing (test harness aid)."""
    if "nc" not in _last:
        return None
    from concourse.bass_utils import run_bass_kernel_spmd
    return run_bass_kernel_spmd(_last["nc"], _last["in_maps"], list(range(8)),
                                trace=True)


def _build_nc(apply_w_q, apply_b_q, apply_w_k, apply_b_k, interleave=True, new_pools=True, fast_recip=True):
    import concourse.bass as bass
    import concourse.mybir as mybir
    import concourse.tile as tile
    from concourse import bacc
    from concourse.bass import ts

    f32 = mybir.dt.float32
    bf16 = mybir.dt.bfloat16
    Exp = mybir.ActivationFunctionType.Exp
    Sqrt = mybir.ActivationFunctionType.Sqrt
    add_op = mybir.AluOpType.add
    mult_op = mybir.AluOpType.mult

    nc = bacc.Bacc()

    hT_d = nc.dram_tensor("hT", [D, SP], bf16, kind="ExternalInput")
    wqT_d = nc.dram_tensor("wqT", [D, FPC], bf16, kind="ExternalInput")
    wkT_d = nc.dram_tensor("wkT", [D, FPC], bf16, kind="ExternalInput")
    wvT_d = nc.dram_tensor("wvT", [D, FPC], bf16, kind="ExternalInput")
    woT_d = nc.dram_tensor("woT", [FPC, D], bf16, kind="ExternalInput")
    bq_d = nc.dram_tensor("bq", [128, 2], f32, kind="ExternalInput")
    bk_d = nc.dram_tensor("bk", [128, 2], f32, kind="ExternalInput")
    bv_d = nc.dram_tensor("bv", [128, FPC], f32, kind="ExternalInput")
    cosT_d = nc.dram_tensor("cosT", [128, S_VID], f32, kind="ExternalInput")
    sinT_d = nc.dram_tensor("sinT", [128, S_VID], f32, kind="ExternalInput")
    rope_d = nc.dram_tensor("ropeT", [128, 128], bf16, kind="ExternalInput")
    onesb_d = nc.dram_tensor("onesblk", [128, 2], bf16, kind="ExternalInput")
    esel_d = nc.dram_tensor("esel", [2, 128], f32, kind="ExternalInput")
    ones64_d = nc.dram_tensor("ones64", [1, 64], f32, kind="ExternalInput")
    lnc_d = nc.dram_tensor("lncols", [128, 4], f32, kind="ExternalInput")
    out_d = nc.dram_tensor("out", [SP, D], f32, kind="ExternalOutput")

    with tile.TileContext(nc) as tc:
        from contextlib import ExitStack
        with ExitStack() as ctx:
            consts = ctx.enter_context(tc.tile_pool(name="consts", bufs=1))
            if new_pools:
                big = ctx.enter_context(tc.tile_pool(name="bigps", bufs=2, space="PSUM"))
                sps = ctx.enter_context(tc.tile_pool(name="sps", bufs=4, space="PSUM"))
            else:
                big = ctx.enter_context(tc.tile_pool(name="bigps", bufs=4, space="PSUM"))
                sps = ctx.enter_context(tc.tile_pool(name="vps", bufs=2, space="PSUM"))
            ops = ctx.enter_context(tc.tile_pool(name="ops", bufs=2, space="PSUM"))
            sbA = ctx.enter_context(tc.tile_pool(name="sbA", bufs=3))
            sbS = ctx.enter_context(tc.tile_pool(name="sbS", bufs=4))
            sbP = ctx.enter_context(tc.tile_pool(name="sbP", bufs=6))
            sbO = ctx.enter_context(tc.tile_pool(name="sbO", bufs=3))

            # ---- resident constants / weights / state ----
            h_sb = [consts.tile([128, SP], bf16, name=f"h{m}", tag=f"h{m}")
                    for m in range(NMC)]
            wq_sb = consts.tile([128, NMC, FPC], bf16)
            wk_sb = consts.tile([128, NMC, FPC], bf16)
            wv_sb = consts.tile([128, NMC, FPC], bf16)
            wo_sb = consts.tile([128, 2, D], bf16)
            cos_sb = consts.tile([128, S_VID], f32)
            sin_sb = consts.tile([128, S_VID], f32)
            rope_sb = consts.tile([128, 128], bf16)
            onesb_sb = consts.tile([128, 2], bf16)
            esel_sb = consts.tile([2, 128], f32)
            ones64_sb = consts.tile([1, 64], f32)
            bq_sb = consts.tile([128, 2], f32)
            bk_sb = consts.tile([128, 2], f32)
            bv_sb = consts.tile([128, HPC, DH], f32)
            ln_sb = consts.tile([128, 4], f32)
            eps_sb = consts.tile([2, 1], f32)
            qT = consts.tile([128, 2, SP], bf16)
            kT = consts.tile([128, 2, SP], bf16)
            vaug = consts.tile([128, NJC, HPC, 66], bf16)
            oT = consts.tile([128, 2, SP], bf16)

            nc.sync.dma_start(wq_sb, wqT_d[:, :].rearrange("(mc p) d -> p mc d", p=128))
            nc.sync.dma_start(wk_sb, wkT_d[:, :].rearrange("(mc p) d -> p mc d", p=128))
            nc.sync.dma_start(wv_sb, wvT_d[:, :].rearrange("(mc p) d -> p mc d", p=128))
            nc.sync.dma_start(wo_sb, woT_d[:, :].rearrange("(c p) n -> p c n", p=128))
            nc.sync.dma_start(cos_sb, cosT_d[:, :])
            nc.sync.dma_start(sin_sb, sinT_d[:, :])
            nc.sync.dma_start(rope_sb, rope_d[:, :])
            nc.sync.dma_start(onesb_sb, onesb_d[:, :])
            nc.sync.dma_start(esel_sb, esel_d[:, :])
            nc.sync.dma_start(ones64_sb, ones64_d[:, :])
            nc.sync.dma_start(bq_sb, bq_d[:, :])
            nc.sync.dma_start(bk_sb, bk_d[:, :])
            nc.sync.dma_start(ln_sb, lnc_d[:, :])
            nc.sync.dma_start(bv_sb, bv_d[:, :].rearrange("p (h d) -> p h d", d=DH))
            for m in range(NMC):
                nc.sync.dma_start(h_sb[m], hT_d[ts(m, 128), :])
            nc.vector.memset(eps_sb, LN_EPS)

            # vaug ones/padding prep (partition bases must be 32-aligned)
            nc.vector.memset(vaug[:, :, :, 64:66], 0.0)
            nc.vector.memset(vaug[96:128, NJC - 1, :, 0:64], 0.0)
            nc.vector.memset(vaug[:, 0:NJC - 1, :, 64:65], 1.0)
            nc.vector.memset(vaug[0:96, NJC - 1, :, 64:65], 1.0)
            nc.vector.memset(vaug[96:VALID_LAST, NJC - 1, :, 64:65], 1.0)

            # ---- phase 1a: v for all key chunks ----
            for ib, (i0, W) in enumerate(IBLKS):
                for sc in range(W // 128):
                    jc = i0 // 128 + sc
                    if new_pools:
                        ps_v = sps.tile([128, 512], f32, tag="s", name="s_ps")[:, 0:FPC]
                    else:
                        ps_v = sps.tile([128, FPC], f32, tag="v", name="ps_v")
                    for m in range(NMC):
                        nc.tensor.matmul(ps_v, h_sb[m][:, i0 + 128 * sc:i0 + 128 * (sc + 1)],
                                         wv_sb[:, m, :], start=(m == 0), stop=(m == NMC - 1))
                    if jc == NJC - 1:
                        # split at 32-aligned partition bases; rows 98+ stay 0
                        nc.vector.tensor_add(
                            vaug[0:96, jc, :, 0:64],
                            ps_v[0:96].rearrange("p (h d) -> p h d", d=DH),
                            bv_sb[0:96])
                        nc.vector.tensor_add(
                            vaug[96:VALID_LAST, jc, :, 0:64],
                            ps_v[96:VALID_LAST].rearrange("p (h d) -> p h d", d=DH),
                            bv_sb[96:VALID_LAST])
                    else:
                        nc.vector.tensor_add(
                            vaug[:, jc, :, 0:64],
                            ps_v[:].rearrange("p (h d) -> p h d", d=DH),
                            bv_sb[:])

            # ---- phase 1b/2: per chunk: q/k production then attention ----
            def qk_block(c, ib, i0, W, wT_sb, b_sb, dstT, apply_w, apply_b, wcol, bcol):
                ps = big.tile([128, 512], f32, tag="big")
                for m in range(NMC):
                    nc.tensor.matmul(ps[:, 0:W], wT_sb[:, m, ts(c, 128)],
                                     h_sb[m][:, i0:i0 + W],
                                     start=(m == 0), stop=(m == NMC - 1))
                xc = sbA.tile([128, 512], bf16, tag="xc", name="xc")[:, 0:W]
                nc.vector.tensor_scalar(xc, ps[:, 0:W], b_sb[:, c:c + 1], None, add_op)
                sq = sbA.tile([128, 512], bf16, tag="sq", name="sq")[:, 0:W]
                nc.vector.tensor_mul(sq, xc, xc)
                st = big.tile([128, 512], f32, tag="big")
                nc.tensor.matmul(st[0:2, 0:W], onesb_sb, sq, start=True, stop=True)
                sd = sbS.tile([2, 512], f32, tag="sd", name="sd")[:, 0:W]
                nc.scalar.activation(sd, st[0:2, 0:W], Sqrt, scale=1.0 / DH, bias=eps_sb)
                rstd = sbS.tile([2, 512], f32, tag="rstd", name="rstd")[:, 0:W]
                if fast_recip:
                    nc.vector.reciprocal_approx_fast(rstd, sd)
                else:
                    nc.vector.reciprocal(rstd, sd)
                rb = big.tile([128, 512], f32, tag="big")
                nc.tensor.matmul(rb[:, 0:W], esel_sb, rstd, start=True, stop=True)
                is_rope = ib < N_VID_IB
                need_wb = apply_w or apply_b
                if is_rope or need_wb:
                    xn = sbA.tile([128, 512], bf16, tag="xn", name="xn")[:, 0:W]
                else:
                    xn = dstT[:, c, i0:i0 + W]
                nc.vector.tensor_mul(xn, xc, rb[:, 0:W])
                if need_wb:
                    tgt = sbA.tile([128, 512], bf16, tag="xw", name="xw")[:, 0:W] if is_rope \
                        else dstT[:, c, i0:i0 + W]
                    if apply_w and apply_b:
                        nc.vector.tensor_scalar(tgt, xn, wcol, bcol, mult_op, add_op)
                    elif apply_w:
                        nc.vector.tensor_scalar(tgt, xn, wcol, None, mult_op)
                    else:
                        nc.vector.tensor_scalar(tgt, xn, bcol, None, add_op)
                    xn = tgt
                if is_rope:
                    rp = big.tile([128, 512], f32, tag="big")
                    nc.tensor.matmul(rp[:, 0:W], rope_sb, xn, start=True, stop=True)
                    t1 = sbA.tile([128, 512], f32, tag="t1", name="t1")[:, 0:W]
                    nc.vector.tensor_mul(t1, xn, cos_sb[:, i0:i0 + W])
                    t2 = sbA.tile([128, 512], f32, tag="t2", name="t2")[:, 0:W]
                    nc.vector.tensor_mul(t2, rp[:, 0:W], sin_sb[:, i0:i0 + W])
                    nc.vector.tensor_add(dstT[:, c, i0:i0 + W], t1, t2)

            def attn_block(c, ib, i0, W):
                o_ps = [ops.tile([65, 512], f32, tag="o", name="o_ps")[:, 0:W] for _ in range(2)]
                for jc in range(NJC):
                    for hh in range(2):
                        p0 = 64 * hh
                        if new_pools:
                            s_ps = sps.tile([128, 512], f32, tag="s", name="s_ps")
                        else:
                            s_ps = big.tile([128, 512], f32, tag="big", name="s_ps")
                        nc.tensor.matmul(s_ps[:, 0:W],
                                         kT[p0:p0 + 64, c, ts(jc, 128)],
                                         qT[p0:p0 + 64, c, i0:i0 + W],
                                         start=True, stop=True,
                                         tile_position=(p0, 0))
                        p_sb = sbP.tile([128, 512], bf16, tag="p", name="p_sb")[:, 0:W]
                        nc.scalar.activation(p_sb, s_ps[:, 0:W], Exp, scale=0.125)
                        nc.tensor.matmul(o_ps[hh], vaug[:, jc, 2 * c + hh, 0:65], p_sb,
                                         start=(jc == 0), stop=(jc == NJC - 1))
                for hh in range(2):
                    osb = sbA.tile([65, 512], f32, tag="osb", name="osb")[:, 0:W]
                    nc.vector.tensor_copy(osb, o_ps[hh])
                    den0 = sbS.tile([1, 512], f32, tag="den0", name="den0")[:, 0:W]
                    nc.vector.tensor_copy(den0, osb[64:65])
                    rden = sbS.tile([1, 512], f32, tag="rden", name="rden")[:, 0:W]
                    if fast_recip:
                        nc.vector.reciprocal_approx_fast(rden, den0)
                    else:
                        nc.vector.reciprocal(rden, den0)
                    db = big.tile([128, 512], f32, tag="big")
                    nc.tensor.matmul(db[0:64, 0:W], ones64_sb, rden, start=True, stop=True)
                    p0 = 64 * hh
                    nc.vector.tensor_mul(oT[p0:p0 + 64, c, i0:i0 + W],
                                         osb[0:64], db[0:64, 0:W])

            def qk_both(c, ib, i0, W):
                qk_block(c, ib, i0, W, wq_sb, bq_sb, qT, apply_w_q, apply_b_q,
                         ln_sb[:, 0:1], ln_sb[:, 1:2])
                qk_block(c, ib, i0, W, wk_sb, bk_sb, kT, apply_w_k, apply_b_k,
                         ln_sb[:, 2:3], ln_sb[:, 3:4])

            INTERLEAVE = interleave
            if INTERLEAVE:
                for ib, (i0, W) in enumerate(IBLKS):
                    qk_both(0, ib, i0, W)
                # chunk-0 attention; chunk-1 q/k emitted in between so its
                # matmuls fill PE gaps while ACT runs exp (keeps HAM warm)
                for ib, (i0, W) in enumerate(IBLKS):
                    attn_block(0, ib, i0, W)
                    qk_both(1, ib, i0, W)
                for ib, (i0, W) in enumerate(IBLKS):
                    attn_block(1, ib, i0, W)
            else:
                for c in range(2):
                    for ib, (i0, W) in enumerate(IBLKS):
                        qk_both(c, ib, i0, W)
                    for ib, (i0, W) in enumerate(IBLKS):
                        attn_block(c, ib, i0, W)

            # ---- phase 3: output projection (partial) ----
            for ic in range(NJC):
                ps0 = big.tile([128, 512], f32, tag="big")
                ps1 = big.tile([128, 512], f32, tag="big")
                for c in range(2):
                    nc.tensor.matmul(ps0, oT[:, c, ts(ic, 128)], wo_sb[:, c, 0:512],
                                     start=(c == 0), stop=(c == 1))
                    nc.tensor.matmul(ps1, oT[:, c, ts(ic, 128)], wo_sb[:, c, 512:1024],
                                     start=(c == 0), stop=(c == 1))
                ob = sbO.tile([128, D], f32, tag="ob")
                nc.vector.tensor_copy(ob[:, 0:512], ps0)
                nc.vector.tensor_copy(ob[:, 512:1024], ps1)
                nc.sync.dma_start(out_d[ts(ic, 128), :], ob)

    nc.finalize()
    return nc


def _get_prog(flags):
    if flags not in _prog_cache:
        _prog_cache[flags] = _build_nc(*flags)
    return _prog_cache[flags]


def _rope_mat():
    P = np.zeros((DH, DH), np.float32)
    for m in range(DH // 2):
        P[2 * m, 2 * m + 1] = -1.0
        P[2 * m + 1, 2 * m] = 1.0
    # lhsT = P^T, block-diagonal over the two heads in a partition chunk
    PT = P.T
    R = np.zeros((128, 128), np.float32)
    R[0:64, 0:64] = PT
    R[64:128, 64:128] = PT
    return R


def kernel(hidden_states, encoder_hidden_states, cos, sin, wq, bq, wk, bk,
           wv, bv, wo, bo, lnq_w, lnq_b, lnk_w, lnk_b):
    from concourse.bass_utils import run_bass_kernel_spmd

    f32 = np.float32
    hs = np.asarray(hidden_states, f32)
    ehs = np.asarray(encoder_hidden_states, f32)
    cos = np.asarray(cos, f32)
    sin = np.asarray(sin, f32)
    wq = np.asarray(wq, f32); bq = np.asarray(bq, f32)
    wk = np.asarray(wk, f32); bk = np.asarray(bk, f32)
    wv = np.asarray(wv, f32); bv = np.asarray(bv, f32)
    wo = np.asarray(wo, f32); bo = np.asarray(bo, f32)
    lnq_w = np.asarray(lnq_w, f32); lnq_b = np.asarray(lnq_b, f32)
    lnk_w = np.asarray(lnk_w, f32); lnk_b = np.asarray(lnk_b, f32)

    flags = (not np.all(lnq_w == 1.0), bool(np.any(lnq_b)),
             not np.all(lnk_w == 1.0), bool(np.any(lnk_b)))
    nc = _get_prog(flags)

    # fold LN mean-centering into wq/wk (per 64-row head block)
    def center_rows(w):
        shp = w.shape
        w64 = w.astype(np.float64).reshape(H, DH, -1)
        w64 = w64 - w64.mean(axis=1, keepdims=True)
        return w64.reshape(shp).astype(f32)

    wq_c = center_rows(wq)
    wk_c = center_rows(wk)
    bq_c = center_rows(bq.reshape(D, 1)).reshape(D)
    bk_c = center_rows(bk.reshape(D, 1)).reshape(D)

    rope = _rope_mat().astype(_BF16)
    onesblk = np.zeros((128, 2), _BF16)
    onesblk[0:64, 0] = 1.0
    onesblk[64:128, 1] = 1.0
    esel = np.zeros((2, 128), f32)
    esel[0, 0:64] = 1.0
    esel[1, 64:128] = 1.0
    ones64 = np.ones((1, 64), f32)
    cosT = np.concatenate([cos.T, cos.T], axis=0).astype(f32).copy()
    sinT = np.concatenate([sin.T, sin.T], axis=0).astype(f32).copy()
    lncols = np.stack([np.tile(lnq_w, 2), np.tile(lnq_b, 2),
                       np.tile(lnk_w, 2), np.tile(lnk_b, 2)], axis=1).astype(f32)

    in_maps = []
    for core in range(8):
        b = core // 4
        g = core % 4
        F = slice(FPC * g, FPC * (g + 1))
        h_full = np.concatenate([hs[b], ehs[b]], axis=0)      # video-first [S, D]
        hT = np.zeros((D, SP), _BF16)
        hT[:, :S] = h_full.T.astype(_BF16)
        in_maps.append({
            "hT": hT,
            "wqT": np.ascontiguousarray(wq_c[F].T).astype(_BF16),
            "wkT": np.ascontiguousarray(wk_c[F].T).astype(_BF16),
            "wvT": np.ascontiguousarray(wv[F].T).astype(_BF16),
            "woT": np.ascontiguousarray(wo[:, F].T).astype(_BF16),
            "bq": np.ascontiguousarray(bq_c[F].reshape(2, 128).T).astype(f32),
            "bk": np.ascontiguousarray(bk_c[F].reshape(2, 128).T).astype(f32),
            "bv": np.ascontiguousarray(np.tile(bv[F], (128, 1))).astype(f32),
            "cosT": cosT, "sinT": sinT,
            "ropeT": rope, "onesblk": onesblk, "esel": esel, "ones64": ones64,
            "lncols": lncols,
        })

    _last["nc"] = nc
    _last["in_maps"] = in_maps
    res = run_bass_kernel_spmd(nc, in_maps, list(range(8)))
    outs = np.zeros((B, SP, D), f32)
    for core in range(8):
        outs[core // 4] += np.asarray(res.results[core]["out"], f32)
    outs += bo[None, None, :]
    video = np.ascontiguousarray(outs[:, :S_VID, :])
    text = np.ascontiguousarray(outs[:, S_VID:S, :])
    return video, text


# revision 17
# speedup vs baseline: 1.5139x; 1.1422x over previous
"""Trainium2 Bass kernel for the CogVideoX attention-processor block.

Sharding (8 cores): core c handles batch c//4 and head-group c%4 (4 of 16
heads). Each core computes QKV projections for its 256 features over the full
(text+video) sequence, per-head LayerNorm + RoPE, full softmax attention for
its 4 heads, and a partial output projection [S, 1024]. The host sums the 4
partials per batch and adds the output bias.

Device-side layout notes:
  - Sequence is laid out video-first ([video 2048 | text 226 | pad 30] = 2304)
    so the RoPE region is exactly the first four 512-wide i-blocks.
  - q/k are produced *transposed* ([dh, S], feature-on-partition) so the
    scores matmul s_T = k @ q^T and the PV matmul contract on partitions.
  - LayerNorm mean-centering is folded into wq/wk on the host (C = I - 1/64
    per head block); on-device LN is sumsq (ones-matmul) -> sqrt -> recip ->
    outer-product broadcast -> multiply.
  - Softmax skips max-subtraction: LN gives q,k exact norm sqrt(64), so
    |scores| <= 8 after the 1/8 scale; exp is computed directly on ACT.
  - The PV matmul uses v augmented with a ones column, yielding the softmax
    denominator as row 64 of the output for free. Padding rows of v_aug are
    zeroed, which masks the 30 pad key positions exactly.
"""

import numpy as np
import ml_dtypes

B, S_TXT, S_VID, D, H = 2, 226, 2048, 1024, 16
DH = 64
S = S_TXT + S_VID            # 2274
SP = 2304                    # padded to 18*128
HPC = 4                      # heads per core
FPC = HPC * DH               # features per core (256)
LN_EPS = 1e-5
NMC = D // 128               # 8 contraction chunks
NJC = SP // 128              # 18 key chunks
IBLKS = [(0, 512), (512, 512), (1024, 512), (1536, 512), (2048, 256)]
N_VID_IB = 4                 # i-blocks 0..3 are exactly the video positions
PAD_ROWS = SP - S            # 30
VALID_LAST = 128 - PAD_ROWS  # 98 valid rows in the last key chunk

_BF16 = ml_dtypes.bfloat16
_prog_cache = {}
_last = {}


def _last_run_for_profile(inputs=None):
    """Re-run the last kernel invocation with NTFF tracing (test harness aid)."""
    if "nc" not in _last:
        return None
    from concourse.bass_utils import run_bass_kernel_spmd
    return run_bass_kernel_spmd(_last["nc"], _last["in_maps"], list(range(8)),
                                trace=True)


def _build_nc(apply_w_q, apply_b_q, apply_w_k, apply_b_k, interleave=True, new_pools=True, fast_recip=True):
    import concourse.bass as bass
    import concourse.mybir as mybir
    import concourse.tile as tile
    from concourse import bacc
    from concourse.bass import ts

    f32 = mybir.dt.float32
    bf16 = mybir.dt.bfloat16
    Exp = mybir.ActivationFunctionType.Exp
    Sqrt = mybir.ActivationFunctionType.Sqrt
    Ln = mybir.ActivationFunctionType.Ln
    add_op = mybir.AluOpType.add
    mult_op = mybir.AluOpType.mult

    nc = bacc.Bacc()

    hT_d = nc.dram_tensor("hT", [D, SP], bf16, kind="ExternalInput")
    wqT_d = nc.dram_tensor("wqT", [D, FPC], bf16, kind="ExternalInput")
    wkT_d = nc.dram_tensor("wkT", [D, FPC], bf16, kind="ExternalInput")
    wvT_d = nc.dram_tensor("wvT", [D, FPC], bf16, kind="ExternalInput")
    woT_d = nc.dram_tensor("woT", [FPC, D], bf16, kind="ExternalInput")
    bq_d = nc.dram_tensor("bq", [128, 2], f32, kind="ExternalInput")
    bk_d = nc.dram_tensor("bk", [128, 2], f32, kind="ExternalInput")
    bv_d = nc.dram_tensor("bv", [128, FPC], f32, kind="ExternalInput")
    cosT_d = nc.dram_tensor("cosT", [128, S_VID], f32, kind="ExternalInput")
    sinT_d = nc.dram_tensor("sinT", [128, S_VID], f32, kind="ExternalInput")
    rope_d = nc.dram_tensor("ropeT", [128, 128], bf16, kind="ExternalInput")
    onesb_d = nc.dram_tensor("onesblk", [128, 2], bf16, kind="ExternalInput")
    esel_d = nc.dram_tensor("esel", [2, 128], f32, kind="ExternalInput")
    ones64_d = nc.dram_tensor("ones64", [1, 64], f32, kind="ExternalInput")
    lnc_d = nc.dram_tensor("lncols", [128, 4], f32, kind="ExternalInput")
    out_d = nc.dram_tensor("out", [SP, D], f32, kind="ExternalOutput")

    with tile.TileContext(nc) as tc:
        from contextlib import ExitStack
        with ExitStack() as ctx:
            consts = ctx.enter_context(tc.tile_pool(name="consts", bufs=1))
            big = ctx.enter_context(tc.tile_pool(name="bigps", bufs=2, space="PSUM"))
            sps = ctx.enter_context(tc.tile_pool(name="sps", bufs=2, space="PSUM"))
            ops = ctx.enter_context(tc.tile_pool(name="ops", bufs=2, space="PSUM"))
            sbA = ctx.enter_context(tc.tile_pool(name="sbA", bufs=3))
            sbS = ctx.enter_context(tc.tile_pool(name="sbS", bufs=4))
            sbP = ctx.enter_context(tc.tile_pool(name="sbP", bufs=6))
            sbO = ctx.enter_context(tc.tile_pool(name="sbO", bufs=3))

            # ---- resident constants / weights / state ----
            h_sb = [consts.tile([128, SP], bf16, name=f"h{m}", tag=f"h{m}")
                    for m in range(NMC)]
            wq_sb = consts.tile([128, NMC, FPC], bf16)
            wk_sb = consts.tile([128, NMC, FPC], bf16)
            wv_sb = consts.tile([128, NMC, FPC], bf16)
            wo_sb = consts.tile([128, 2, D], bf16)
            cos_sb = consts.tile([128, S_VID], f32)
            sin_sb = consts.tile([128, S_VID], f32)
            rope_sb = consts.tile([128, 128], bf16)
            onesb_sb = consts.tile([128, 2], bf16)
            esel_sb = consts.tile([2, 128], f32)
            ones64_sb = consts.tile([1, 64], f32)
            bq_sb = consts.tile([128, 2], f32)
            bk_sb = consts.tile([128, 2], f32)
            bv_sb = consts.tile([128, HPC, DH], f32)
            ln_sb = consts.tile([128, 4], f32)
            eps_sb = consts.tile([2, 1], f32)
            qT = consts.tile([128, 2, SP], bf16)
            kT = consts.tile([128, 2, SP], bf16)
            vaug = consts.tile([128, NJC, HPC, 66], bf16)
            oT = consts.tile([128, 2, SP], bf16)

            nc.sync.dma_start(wq_sb, wqT_d[:, :].rearrange("(mc p) d -> p mc d", p=128))
            nc.sync.dma_start(wk_sb, wkT_d[:, :].rearrange("(mc p) d -> p mc d", p=128))
            nc.sync.dma_start(wv_sb, wvT_d[:, :].rearrange("(mc p) d -> p mc d", p=128))
            nc.sync.dma_start(wo_sb, woT_d[:, :].rearrange("(c p) n -> p c n", p=128))
            nc.sync.dma_start(cos_sb, cosT_d[:, :])
            nc.sync.dma_start(sin_sb, sinT_d[:, :])
            nc.sync.dma_start(rope_sb, rope_d[:, :])
            nc.sync.dma_start(onesb_sb, onesb_d[:, :])
            nc.sync.dma_start(esel_sb, esel_d[:, :])
            nc.sync.dma_start(ones64_sb, ones64_d[:, :])
            nc.sync.dma_start(bq_sb, bq_d[:, :])
            nc.sync.dma_start(bk_sb, bk_d[:, :])
            nc.sync.dma_start(ln_sb, lnc_d[:, :])
            nc.sync.dma_start(bv_sb, bv_d[:, :].rearrange("p (h d) -> p h d", d=DH))
            for m in range(NMC):
                nc.sync.dma_start(h_sb[m], hT_d[ts(m, 128), :])
            nc.vector.memset(eps_sb, LN_EPS)

            # vaug ones/padding prep (partition bases must be 32-aligned)
            nc.vector.memset(vaug[:, :, :, 64:66], 0.0)
            nc.vector.memset(vaug[96:128, NJC - 1, :, 0:64], 0.0)
            nc.vector.memset(vaug[:, 0:NJC - 1, :, 64:65], 1.0)
            nc.vector.memset(vaug[0:96, NJC - 1, :, 64:65], 1.0)
            nc.vector.memset(vaug[96:VALID_LAST, NJC - 1, :, 64:65], 1.0)

            # ---- phase 1a: v for all key chunks ----
            for ib, (i0, W) in enumerate(IBLKS):
                for sc in range(W // 128):
                    jc = i0 // 128 + sc
                    ps_v = sps.tile([128, 2, 512], f32, tag="s2", name="s2")[:, 0, 0:FPC]
                    for m in range(NMC):
                        nc.tensor.matmul(ps_v, h_sb[m][:, i0 + 128 * sc:i0 + 128 * (sc + 1)],
                                         wv_sb[:, m, :], start=(m == 0), stop=(m == NMC - 1))
                    if jc == NJC - 1:
                        # split at 32-aligned partition bases; rows 98+ stay 0
                        nc.vector.tensor_add(
                            vaug[0:96, jc, :, 0:64],
                            ps_v[0:96].rearrange("p (h d) -> p h d", d=DH),
                            bv_sb[0:96])
                        nc.vector.tensor_add(
                            vaug[96:VALID_LAST, jc, :, 0:64],
                            ps_v[96:VALID_LAST].rearrange("p (h d) -> p h d", d=DH),
                            bv_sb[96:VALID_LAST])
                    else:
                        nc.vector.tensor_add(
                            vaug[:, jc, :, 0:64],
                            ps_v[:].rearrange("p (h d) -> p h d", d=DH),
                            bv_sb[:])

            # ---- phase 1b/2: per chunk: q/k production then attention ----
            def qk_block(c, ib, i0, W, wT_sb, b_sb, dstT, apply_w, apply_b, wcol, bcol):
                ps = big.tile([128, 512], f32, tag="big")
                for m in range(NMC):
                    nc.tensor.matmul(ps[:, 0:W], wT_sb[:, m, ts(c, 128)],
                                     h_sb[m][:, i0:i0 + W],
                                     start=(m == 0), stop=(m == NMC - 1))
                xc = sbA.tile([128, 512], bf16, tag="xc", name="xc")[:, 0:W]
                nc.vector.tensor_scalar(xc, ps[:, 0:W], b_sb[:, c:c + 1], None, add_op)
                sq = sbA.tile([128, 512], bf16, tag="sq", name="sq")[:, 0:W]
                nc.vector.tensor_mul(sq, xc, xc)
                st = big.tile([128, 512], f32, tag="big")
                nc.tensor.matmul(st[0:2, 0:W], onesb_sb, sq, start=True, stop=True)
                lg = sbS.tile([2, 512], f32, tag="sd", name="lg")[:, 0:W]
                nc.scalar.activation(lg, st[0:2, 0:W], Ln, scale=1.0 / DH, bias=eps_sb)
                rstd = sbS.tile([2, 512], f32, tag="rstd", name="rstd")[:, 0:W]
                nc.scalar.activation(rstd, lg, Exp, scale=-0.5)
                rb = big.tile([128, 512], f32, tag="big")
                nc.tensor.matmul(rb[:, 0:W], esel_sb, rstd, start=True, stop=True)
                is_rope = ib < N_VID_IB
                need_wb = apply_w or apply_b
                if is_rope or need_wb:
                    xn = sbA.tile([128, 512], bf16, tag="xn", name="xn")[:, 0:W]
                else:
                    xn = dstT[:, c, i0:i0 + W]
                nc.vector.tensor_mul(xn, xc, rb[:, 0:W])
                if need_wb:
                    tgt = sbA.tile([128, 512], bf16, tag="xw", name="xw")[:, 0:W] if is_rope \
                        else dstT[:, c, i0:i0 + W]
                    if apply_w and apply_b:
                        nc.vector.tensor_scalar(tgt, xn, wcol, bcol, mult_op, add_op)
                    elif apply_w:
                        nc.vector.tensor_scalar(tgt, xn, wcol, None, mult_op)
                    else:
                        nc.vector.tensor_scalar(tgt, xn, bcol, None, add_op)
                    xn = tgt
                if is_rope:
                    rp = big.tile([128, 512], f32, tag="big")
                    nc.tensor.matmul(rp[:, 0:W], rope_sb, xn, start=True, stop=True)
                    t1 = sbA.tile([128, 512], f32, tag="t1", name="t1")[:, 0:W]
                    nc.vector.tensor_mul(t1, xn, cos_sb[:, i0:i0 + W])
                    t2 = sbA.tile([128, 512], f32, tag="t2", name="t2")[:, 0:W]
                    nc.vector.tensor_mul(t2, rp[:, 0:W], sin_sb[:, i0:i0 + W])
                    nc.vector.tensor_add(dstT[:, c, i0:i0 + W], t1, t2)

            def attn_block(c, ib, i0, W):
                o_ps = [ops.tile([65, 512], f32, tag="o", name="o_ps")[:, 0:W] for _ in range(2)]
                for jc in range(NJC):
                    s2 = sps.tile([128, 2, 512], f32, tag="s2", name="s2")
                    for hh in range(2):
                        p0 = 64 * hh
                        nc.tensor.matmul(s2[:, hh, 0:W],
                                         kT[p0:p0 + 64, c, ts(jc, 128)],
                                         qT[p0:p0 + 64, c, i0:i0 + W],
                                         start=True, stop=True,
                                         tile_position=(p0, 0))
                    p2 = sbP.tile([128, 2, 512], bf16, tag="p", name="p2")
                    nc.scalar.activation(p2[:, :, 0:W], s2[:, :, 0:W], Exp, scale=0.125)
                    for hh in range(2):
                        nc.tensor.matmul(o_ps[hh], vaug[:, jc, 2 * c + hh, 0:65],
                                         p2[:, hh, 0:W],
                                         start=(jc == 0), stop=(jc == NJC - 1))
                for hh in range(2):
                    osb = sbA.tile([65, 512], f32, tag="osb", name="osb")[:, 0:W]
                    nc.vector.tensor_copy(osb, o_ps[hh])
                    den0 = sbS.tile([1, 512], f32, tag="den0", name="den0")[:, 0:W]
                    nc.vector.tensor_copy(den0, osb[64:65])
                    rden = sbS.tile([1, 512], f32, tag="rden", name="rden")[:, 0:W]
                    if fast_recip:
                        nc.vector.reciprocal_approx_fast(rden, den0)
                    else:
                        nc.vector.reciprocal(rden, den0)
                    db = big.tile([128, 512], f32, tag="big")
                    nc.tensor.matmul(db[0:64, 0:W], ones64_sb, rden, start=True, stop=True)
                    p0 = 64 * hh
                    nc.vector.tensor_mul(oT[p0:p0 + 64, c, i0:i0 + W],
                                         osb[0:64], db[0:64, 0:W])

            def qk_both(c, ib, i0, W):
                qk_block(c, ib, i0, W, wq_sb, bq_sb, qT, apply_w_q, apply_b_q,
                         ln_sb[:, 0:1], ln_sb[:, 1:2])
                qk_block(c, ib, i0, W, wk_sb, bk_sb, kT, apply_w_k, apply_b_k,
                         ln_sb[:, 2:3], ln_sb[:, 3:4])

            INTERLEAVE = interleave
            if INTERLEAVE:
                for ib, (i0, W) in enumerate(IBLKS):
                    qk_both(0, ib, i0, W)
                # chunk-0 attention; chunk-1 q/k emitted in between so its
                # matmuls fill PE gaps while ACT runs exp (keeps HAM warm)
                for ib, (i0, W) in enumerate(IBLKS):
                    attn_block(0, ib, i0, W)
                    qk_both(1, ib, i0, W)
                for ib, (i0, W) in enumerate(IBLKS):
                    attn_block(1, ib, i0, W)
            else:
                for c in range(2):
                    for ib, (i0, W) in enumerate(IBLKS):
                        qk_both(c, ib, i0, W)
                    for ib, (i0, W) in enumerate(IBLKS):
                        attn_block(c, ib, i0, W)

            # ---- phase 3: output projection (partial) ----
            for ic in range(NJC):
                ps0 = big.tile([128, 512], f32, tag="big")
                ps1 = big.tile([128, 512], f32, tag="big")
                for c in range(2):
                    nc.tensor.matmul(ps0, oT[:, c, ts(ic, 128)], wo_sb[:, c, 0:512],
                                     start=(c == 0), stop=(c == 1))
                    nc.tensor.matmul(ps1, oT[:, c, ts(ic, 128)], wo_sb[:, c, 512:1024],
                                     start=(c == 0), stop=(c == 1))
                ob = sbO.tile([128, D], f32, tag="ob")
                nc.vector.tensor_copy(ob[:, 0:512], ps0)
                nc.vector.tensor_copy(ob[:, 512:1024], ps1)
                nc.sync.dma_start(out_d[ts(ic, 128), :], ob)

    nc.finalize()
    return nc


def _get_prog(flags):
    if flags not in _prog_cache:
        _prog_cache[flags] = _build_nc(*flags)
    return _prog_cache[flags]


def _rope_mat():
    P = np.zeros((DH, DH), np.float32)
    for m in range(DH // 2):
        P[2 * m, 2 * m + 1] = -1.0
        P[2 * m + 1, 2 * m] = 1.0
    # lhsT = P^T, block-diagonal over the two heads in a partition chunk
    PT = P.T
    R = np.zeros((128, 128), np.float32)
    R[0:64, 0:64] = PT
    R[64:128, 64:128] = PT
    return R


def kernel(hidden_states, encoder_hidden_states, cos, sin, wq, bq, wk, bk,
           wv, bv, wo, bo, lnq_w, lnq_b, lnk_w, lnk_b):
    from concourse.bass_utils import run_bass_kernel_spmd

    f32 = np.float32
    hs = np.asarray(hidden_states, f32)
    ehs = np.asarray(encoder_hidden_states, f32)
    cos = np.asarray(cos, f32)
    sin = np.asarray(sin, f32)
    wq = np.asarray(wq, f32); bq = np.asarray(bq, f32)
    wk = np.asarray(wk, f32); bk = np.asarray(bk, f32)
    wv = np.asarray(wv, f32); bv = np.asarray(bv, f32)
    wo = np.asarray(wo, f32); bo = np.asarray(bo, f32)
    lnq_w = np.asarray(lnq_w, f32); lnq_b = np.asarray(lnq_b, f32)
    lnk_w = np.asarray(lnk_w, f32); lnk_b = np.asarray(lnk_b, f32)

    flags = (not np.all(lnq_w == 1.0), bool(np.any(lnq_b)),
             not np.all(lnk_w == 1.0), bool(np.any(lnk_b)))
    nc = _get_prog(flags)

    # fold LN mean-centering into wq/wk (per 64-row head block)
    def center_rows(w):
        shp = w.shape
        w64 = w.astype(np.float64).reshape(H, DH, -1)
        w64 = w64 - w64.mean(axis=1, keepdims=True)
        return w64.reshape(shp).astype(f32)

    wq_c = center_rows(wq)
    wk_c = center_rows(wk)
    bq_c = center_rows(bq.reshape(D, 1)).reshape(D)
    bk_c = center_rows(bk.reshape(D, 1)).reshape(D)

    rope = _rope_mat().astype(_BF16)
    onesblk = np.zeros((128, 2), _BF16)
    onesblk[0:64, 0] = 1.0
    onesblk[64:128, 1] = 1.0
    esel = np.zeros((2, 128), f32)
    esel[0, 0:64] = 1.0
    esel[1, 64:128] = 1.0
    ones64 = np.ones((1, 64), f32)
    cosT = np.concatenate([cos.T, cos.T], axis=0).astype(f32).copy()
    sinT = np.concatenate([sin.T, sin.T], axis=0).astype(f32).copy()
    lncols = np.stack([np.tile(lnq_w, 2), np.tile(lnq_b, 2),
                       np.tile(lnk_w, 2), np.tile(lnk_b, 2)], axis=1).astype(f32)

    in_maps = []
    for core in range(8):
        b = core // 4
        g = core % 4
        F = slice(FPC * g, FPC * (g + 1))
        h_full = np.concatenate([hs[b], ehs[b]], axis=0)      # video-first [S, D]
        hT = np.zeros((D, SP), _BF16)
        hT[:, :S] = h_full.T.astype(_BF16)
        in_maps.append({
            "hT": hT,
            "wqT": np.ascontiguousarray(wq_c[F].T).astype(_BF16),
            "wkT": np.ascontiguousarray(wk_c[F].T).astype(_BF16),
            "wvT": np.ascontiguousarray(wv[F].T).astype(_BF16),
            "woT": np.ascontiguousarray(wo[:, F].T).astype(_BF16),
            "bq": np.ascontiguousarray(bq_c[F].reshape(2, 128).T).astype(f32),
            "bk": np.ascontiguousarray(bk_c[F].reshape(2, 128).T).astype(f32),
            "bv": np.ascontiguousarray(np.tile(bv[F], (128, 1))).astype(f32),
            "cosT": cosT, "sinT": sinT,
            "ropeT": rope, "onesblk": onesblk, "esel": esel, "ones64": ones64,
            "lncols": lncols,
        })

    _last["nc"] = nc
    _last["in_maps"] = in_maps
    res = run_bass_kernel_spmd(nc, in_maps, list(range(8)))
    outs = np.zeros((B, SP, D), f32)
    for core in range(8):
        outs[core // 4] += np.asarray(res.results[core]["out"], f32)
    outs += bo[None, None, :]
    video = np.ascontiguousarray(outs[:, :S_VID, :])
    text = np.ascontiguousarray(outs[:, S_VID:S, :])
    return video, text
